# revision 23
# baseline (speedup 1.0000x reference)
"""Trainium2 Bass kernel for nn_Critic (branch MLPs -> 255-step LSTM -> head).

Strategy (hardcoded, 8 cores, data-parallel over batch B=512 -> 64/core):
  - Everything feature-major on chip: vectors are [feature_chunk(128), batch(64)].
  - bf16 matmul inputs and gate/cell elementwise (2x DVE modes; numpy-sim rel
    err 0.006 vs 2e-2 budget); fp32 PSUM.
  - PSUM z[p, slot, col]: slot = t mod Z_SLOTS holds step t's gate
    pre-activation z_t^T [1024, 64] as 8 m-chunks of 64 cols. Gate order
    i,f,g,o: z_if tile holds m0-3 (i,f), z_go holds m4-7 (g,o) so
    sigmoid(i,f) only waits on the first 8 recurrent matmuls.
  - start=True zeroes the containing 2KB PSUM bank; slots pair up (2j, 2j+1)
    per bank, so exactly one writer per bank carries start=True (first
    m-chunk, k=0, even step).
  - zx = Wk^T x_t precomputed off the critical path: step t emits the 16
    64-col zx matmuls for step t+ZX_LEAD, whose slot was last read at step
    t-2 -> the in-order PE queue never stalls on the WAR dep.
  - PE_HAM clock gate: the PE runs at 1.2 GHz unless near-continuously busy
    (then 2.4 GHz). Dummy 512-col matmuls into 2 scratch PSUM banks fill the
    gate-chain window each step, locking K=8/8 for the whole loop.
  - Critical chain per step (~2.5us, at per-op floors): 8 rec mm ->
    sigmoid(i,f) -> tanh(g) -> DVE [tm2 = sig_f*c, tm1 = sig_i*tanh_g,
    c = tm2+tm1] -> tanh(c) -> h = sig_o*tanh_c -> next rec mms.
"""

import os
os.environ.setdefault("TILE_EXHAUSTIVE_MEMORY_SHARE_CHECK", "1")

import numpy as np
import ml_dtypes

import concourse.bass as bass
import concourse.mybir as mybir
import concourse.tile as tile
from concourse import bacc
from concourse.bass_utils import run_bass_kernel_spmd

BF16 = mybir.dt.bfloat16
F32 = mybir.dt.float32
AF = mybir.ActivationFunctionType

NC = 8          # cores
B = 512
BC = B // NC    # 64 batch per core
T = 255         # real steps
TP = 256        # padded steps
U = 256
DIN = 256

Z_SLOTS = 6      # z PSUM slots in rotation (pairs share a 2KB bank)
ZX_LEAD = Z_SLOTS - 2   # zx for step t+ZX_LEAD emitted at step t
FILLER_MM = 5    # 512-col dummy matmuls per step into scratch PSUM

# packed bf16 constant block: column offsets in t_pkb [128, PKB_COLS]
_PK = {}
_off = 0
for _name, _w in [("mot", BC), ("rob", BC), ("re", BC), ("im", BC),
                  ("wm", 256), ("wr", 256), ("wre", 128), ("wim", 128),
                  ("wc", 6 * 256), ("wo", 2), ("blw", 128), ("ind", 512)]:
    _PK[_name] = _off
    _off += _w
PKB_COLS = _off


def build_nc(use_bias_mm=True):
    nc = bacc.Bacc(None, target_bir_lowering=False)

    d_pkb = nc.dram_tensor("pkb", [128, PKB_COLS], BF16, kind="ExternalInput")
    d_pkf = nc.dram_tensor("pkf", [128, 9], F32, kind="ExternalInput")
    d_seq = nc.dram_tensor("seq", [2, 128, TP * BC], BF16, kind="ExternalInput")
    d_wk = nc.dram_tensor("wk", [128, 2, 1024], BF16, kind="ExternalInput")
    d_wrk = nc.dram_tensor("wrk", [128, 2, 1024], BF16, kind="ExternalInput")
    d_y = nc.dram_tensor("y", [1, BC], F32, kind="ExternalOutput")

    with tile.TileContext(nc) as tc:
        with (
            tc.tile_pool(name="sb", bufs=1) as sb,
            tc.tile_pool(name="rot", bufs=3) as rot,
        ):
            t_pkb = sb.tile([128, PKB_COLS], BF16, tag="pkb")
            t_pkf = sb.tile([128, 9], F32, tag="pkf")
            t_wk = sb.tile([128, 2, 1024], BF16, tag="wk")
            t_wrk = sb.tile([128, 2, 1024], BF16, tag="wrk")
            t_seq0 = sb.tile([128, TP * BC], BF16, tag="seq0")
            t_seq1 = sb.tile([128, TP * BC], BF16, tag="seq1")
            t_h = sb.tile([128, 2 * BC], BF16, tag="h")   # h^T (chunk k at cols k*64)
            t_c = sb.tile([128, 2 * BC], BF16, tag="c")   # c^T
            t_cat = sb.tile([128, 6, BC], BF16, tag="cat")
            t_y = sb.tile([1, BC], F32, tag="y")
            t_dm = sb.tile([1, 1], F32, tag="dm")

            def pkb(name, w, rows=128):
                c = _PK[name]
                return t_pkb[0:rows, c:c + w]

            nc.sync.dma_start(t_pkb[:], d_pkb[:])
            nc.sync.dma_start(t_pkf[:], d_pkf[:])
            nc.sync.dma_start(t_wk[:], d_wk[:])
            nc.sync.dma_start(t_wrk[:], d_wrk[:])
            CH = 64 * BC
            for ch in range(TP // 64):
                nc.sync.dma_start(
                    t_seq0[:, ch * CH:(ch + 1) * CH], d_seq[0, :, ch * CH:(ch + 1) * CH])
                nc.sync.dma_start(
                    t_seq1[:, ch * CH:(ch + 1) * CH], d_seq[1, :, ch * CH:(ch + 1) * CH])
            t_seq = [t_seq0, t_seq1]
            # trigger the sigmoid/tanh ACT table load during the DMA phase
            # (the only table set used: every relu below runs on DVE)
            nc.scalar.activation(t_dm[:], t_pkf[0:1, 0:1], AF.Sigmoid)

            # ---- front-end branch MLPs -> state -> h0, c0 ----
            with tc.tile_pool(name="fp", bufs=1, space="PSUM") as fp:
                p6 = fp.tile([128, 6, BC], F32, tag="p6")
                for m in range(2):
                    nc.tensor.matmul(p6[:, m, :],
                                     pkb("wm", 256, 64)[:, m * 128:(m + 1) * 128],
                                     pkb("mot", BC, 64), start=True, stop=True)
                for m in range(2):
                    nc.tensor.matmul(p6[:, 2 + m, :],
                                     pkb("wr", 256)[:, m * 128:(m + 1) * 128],
                                     pkb("rob", BC), start=True, stop=True)
                nc.tensor.matmul(p6[:, 4, :], pkb("wre", 128), pkb("re", BC),
                                 start=True, stop=True)
                nc.tensor.matmul(p6[:, 5, :], pkb("wim", 128), pkb("im", BC),
                                 start=True, stop=True)
                A = mybir.AluOpType
                fsc = fp.tile([128, 2, 512], F32, tag="fsc")
                for j in range(12):
                    nc.tensor.matmul(fsc[:, j % 2, :],
                                     t_wk[:, 0, 0:128], t_seq0[:, 0:512],
                                     start=True, stop=True,
                                     skip_group_check=True)
                for m in range(6):
                    nc.vector.tensor_scalar(t_cat[:, m, :], p6[:, m, :],
                                            t_pkf[:, m:m + 1], 0.0,
                                            A.add, A.max)
                pst = fp.tile([128, 2, BC], F32, tag="pst")
                for mo in range(2):
                    for kc in range(6):
                        nc.tensor.matmul(
                            pst[:, mo, :],
                            pkb("wc", 1536)[:, kc * 256 + mo * 128:kc * 256 + (mo + 1) * 128],
                            t_cat[:, kc, :],
                            start=(kc == 0), stop=(kc == 5))
                for j in range(12):
                    nc.tensor.matmul(fsc[:, j % 2, :],
                                     t_wk[:, 0, 0:128], t_seq0[:, 0:512],
                                     start=True, stop=True,
                                     skip_group_check=True)
                for mo in range(2):
                    nc.vector.tensor_scalar(t_h[:, mo * BC:(mo + 1) * BC],
                                            pst[:, mo, :],
                                            t_pkf[:, 6 + mo:7 + mo], 0.0,
                                            A.add, A.max)
                    nc.vector.tensor_scalar(t_c[:, mo * BC:(mo + 1) * BC],
                                            pst[:, mo, :],
                                            t_pkf[:, 6 + mo:7 + mo], 0.0,
                                            A.add, A.max)

            # ---- LSTM recurrence ----
            with tc.tile_pool(name="zp", bufs=1, space="PSUM") as zp:
                z_if = zp.tile([128, Z_SLOTS, 4 * BC], F32, tag="zif")
                z_go = zp.tile([128, Z_SLOTS, 4 * BC], F32, tag="zgo")
                scr = zp.tile([128, 2, 512], F32, tag="scr")

                def zslice(bk, m):
                    zt = z_if if m < 4 else z_go
                    mm = m % 4
                    return zt[:, bk, mm * BC:(mm + 1) * BC]

                def emit_bias1(s):
                    bk = s % Z_SLOTS
                    st = (s % 2 == 0)
                    nc.tensor.matmul(z_if[:, bk, :], pkb("blw", 128, 8),
                                     pkb("ind", 512, 8)[:, 0:256],
                                     start=st, stop=False, skip_group_check=True)
                    nc.tensor.matmul(z_go[:, bk, :], pkb("blw", 128, 8),
                                     pkb("ind", 512, 8)[:, 256:512],
                                     start=st, stop=False, skip_group_check=True)

                def emit_zx1(s, ms):
                    bk = s % Z_SLOTS
                    for m in ms:
                        for k in range(2):
                            st = ((not use_bias_mm) and k == 0
                                  and m in (0, 4) and s % 2 == 0)
                            nc.tensor.matmul(
                                zslice(bk, m),
                                t_wk[:, k, m * 128:(m + 1) * 128],
                                t_seq[k][:, s * BC:(s + 1) * BC],
                                start=st, stop=False,
                                skip_group_check=True)

                def emit_step(t):
                    bk = t % Z_SLOTS
                    # i,f m-chunks first so sigmoid(i,f) fires after 8 matmuls
                    for m in range(4):
                        for k in range(2):
                            nc.tensor.matmul(
                                zslice(bk, m),
                                t_wrk[:, k, m * 128:(m + 1) * 128],
                                t_h[:, k * BC:(k + 1) * BC],
                                start=False,
                                stop=(k == 1 and m == 3),
                                skip_group_check=True)
                    for m in range(4, 8):
                        for k in range(2):
                            nc.tensor.matmul(
                                zslice(bk, m),
                                t_wrk[:, k, m * 128:(m + 1) * 128],
                                t_h[:, k * BC:(k + 1) * BC],
                                start=False,
                                stop=(k == 1 and m == 7),
                                skip_group_check=True)
                    gb = rot.tile([128, 512], BF16, tag="gb")
                    tm1 = rot.tile([128, 128], BF16, tag="tm1")
                    tm2 = rot.tile([128, 128], BF16, tag="tm2")
                    tmc = rot.tile([128, 128], BF16, tag="tmc")
                    nc.scalar.activation(gb[:, 0:256], z_if[:, bk, :], AF.Sigmoid)
                    nc.scalar.activation(gb[:, 256:384], z_go[:, bk, 0:128], AF.Tanh)
                    nc.scalar.activation(gb[:, 384:512], z_go[:, bk, 128:256], AF.Sigmoid)
                    # c = sig(f)*c + sig(i)*tanh(g)
                    nc.vector.tensor_mul(tm2[:], gb[:, 128:256], t_c[:])
                    nc.vector.tensor_mul(tm1[:], gb[:, 0:128], gb[:, 256:384])
                    nc.vector.tensor_add(t_c[:], tm2[:], tm1[:])
                    nc.scalar.activation(tmc[:], t_c[:], AF.Tanh)
                    nc.vector.tensor_mul(t_h[:], gb[:, 384:512], tmc[:])

                # preload the zx pipeline
                for s in range(ZX_LEAD):
                    if use_bias_mm:
                        emit_bias1(s)
                    emit_zx1(s, range(8))
                for t in range(T):
                    emit_step(t)
                    # trickle zx for step t+ZX_LEAD (slot read at step t-2)
                    s = t + ZX_LEAD
                    if s < TP:
                        if use_bias_mm:
                            emit_bias1(s)
                        emit_zx1(s, range(8))
                    # PE_HAM un-throttles only under sustained MM activity;
                    # dummy 512-col matmuls into scratch PSUM fill the
                    # gate-chain window so the PE holds K=8/8 (2.4 GHz).
                    for j in range(FILLER_MM):
                        nc.tensor.matmul(scr[:, j % 2, :],
                                         t_wk[:, 0, 0:128], t_seq0[:, 0:512],
                                         start=True, stop=True,
                                         skip_group_check=True)

            # ---- output head ----
            with tc.tile_pool(name="hp", bufs=1, space="PSUM") as hp:
                py = hp.tile([1, BC], F32, tag="py")
                for k in range(2):
                    nc.tensor.matmul(py[:], pkb("wo", 2)[:, k:k + 1],
                                     t_h[:, k * BC:(k + 1) * BC],
                                     start=(k == 0), stop=(k == 1))
                nc.vector.tensor_scalar(t_y[:], py[:],
                                        t_pkf[0:1, 8:9], 0.0,
                                        mybir.AluOpType.add,
                                        mybir.AluOpType.max)
            nc.sync.dma_start(d_y[:], t_y[:])

    nc.compile()
    return nc


_NC_CACHE = None


def _prep_inputs(inputs):
    """Shard + lay out the full-problem inputs into 8 per-core in_maps."""
    bf = ml_dtypes.bfloat16
    f32 = np.float32

    hist = np.asarray(inputs["history"], f32)     # [B, 128, 256]
    act = np.asarray(inputs["action"], f32)       # [B, 128, 256]
    seq = np.concatenate([hist[:, :127], act], axis=1)          # [B, 255, 256]
    seq = np.concatenate(
        [seq, np.zeros((B, 1, DIN), f32)], axis=1)              # [B, 256, 256]

    Wk = np.asarray(inputs["Wk"], f32)            # [256, 1024]
    Wrk = np.asarray(inputs["Wrk"], f32)
    bl = np.asarray(inputs["bl"], f32)            # [1024]
    wk_p = np.ascontiguousarray(
        Wk.reshape(2, 128, 1024).transpose(1, 0, 2)).astype(bf)   # [128,2,1024]
    wrk_p = np.ascontiguousarray(
        Wrk.reshape(2, 128, 1024).transpose(1, 0, 2)).astype(bf)

    # packed bf16 constants (shared across cores except the 4 input slices)
    pkb = np.zeros((128, PKB_COLS), f32)

    def put(name, arr, rows=None):
        a = np.asarray(arr, f32)
        r, w = a.shape
        pkb[0:r, _PK[name]:_PK[name] + w] = a

    put("wm", inputs["Wm"])
    put("wr", inputs["Wr"])
    put("wre", inputs["Wre"])
    put("wim", inputs["Wim"])
    Wc = np.asarray(inputs["Wc"], f32)            # [768, 256] -> [128, 6*256]
    put("wc", np.ascontiguousarray(
        Wc.reshape(6, 128, 256).transpose(1, 0, 2)).reshape(128, 1536))
    Wo = np.asarray(inputs["Wo"], f32)            # [256, 1] -> [128, 2]
    put("wo", np.ascontiguousarray(Wo.reshape(2, 128).T))
    put("blw", bl.reshape(8, 128))
    ind8 = np.zeros((8, 512), f32)
    for j in range(8):
        ind8[j, j * 64:(j + 1) * 64] = 1.0
    put("ind", ind8)

    # packed f32 biases: [bm(2), br(2), bre(1), bim(1), bc(2), bo(1)]
    pkf = np.zeros((128, 9), f32)
    pkf[:, 0:2] = np.asarray(inputs["bm"], f32).reshape(2, 128).T
    pkf[:, 2:4] = np.asarray(inputs["br"], f32).reshape(2, 128).T
    pkf[:, 4:5] = np.asarray(inputs["bre"], f32).reshape(1, 128).T
    pkf[:, 5:6] = np.asarray(inputs["bim"], f32).reshape(1, 128).T
    pkf[:, 6:8] = np.asarray(inputs["bc"], f32).reshape(2, 128).T
    pkf[0, 8] = np.asarray(inputs["bo"], f32).ravel()[0]

    mot = np.asarray(inputs["motion_state"], f32)
    rob = np.asarray(inputs["robot_state"], f32)
    real = np.concatenate([np.asarray(inputs["osc_state_real"], f32),
                           np.asarray(inputs["osc_real"], f32)], -1)
    imag = np.concatenate([np.asarray(inputs["osc_state_imag"], f32),
                           np.asarray(inputs["osc_imag"], f32)], -1)

    in_maps = []
    for c in range(NC):
        sl = slice(c * BC, (c + 1) * BC)
        # on-chip col = t*64 + b  (plain t-major)
        sc = seq[sl].reshape(BC, TP, 2, 128)           # [b, t, fk, fp]
        sc = np.ascontiguousarray(sc.transpose(2, 3, 1, 0)).astype(bf)
        pk = pkb.copy()
        pk[0:64, _PK["mot"]:_PK["mot"] + BC] = mot[sl].T
        pk[:, _PK["rob"]:_PK["rob"] + BC] = rob[sl].T
        pk[:, _PK["re"]:_PK["re"] + BC] = real[sl].T
        pk[:, _PK["im"]:_PK["im"] + BC] = imag[sl].T
        m = {
            "pkb": pk.astype(bf),
            "pkf": pkf,
            "wk": wk_p, "wrk": wrk_p,
            "seq": np.ascontiguousarray(sc.reshape(2, 128, TP * BC)),
        }
        in_maps.append(m)
    return in_maps


def kernel(**inputs):
    global _NC_CACHE
    use_bias_mm = bool(np.any(np.asarray(inputs["bl"])))
    if _NC_CACHE is None or _NC_CACHE[1] != use_bias_mm:
        _NC_CACHE = (build_nc(use_bias_mm), use_bias_mm)
    in_maps = _prep_inputs(inputs)
    res = run_bass_kernel_spmd(_NC_CACHE[0], in_maps, core_ids=list(range(NC)))
    out = np.concatenate(
        [np.asarray(res.results[c]["y"], np.float32).T for c in range(NC)], axis=0)
    return out  # [512, 1] float32


# revision 24
# speedup vs baseline: 1.0126x; 1.0126x over previous
"""Trainium2 Bass kernel for nn_Critic (branch MLPs -> 255-step LSTM -> head).

Strategy (hardcoded, 8 cores, data-parallel over batch B=512 -> 64/core):
  - Everything feature-major on chip: vectors are [feature_chunk(128), batch(64)].
  - bf16 matmul inputs and gate/cell elementwise (2x DVE modes; numpy-sim rel
    err 0.006 vs 2e-2 budget); fp32 PSUM.
  - PSUM z[p, slot, col]: slot = t mod Z_SLOTS holds step t's gate
    pre-activation z_t^T [1024, 64] as 8 m-chunks of 64 cols. Gate order
    i,f,g,o: z_if tile holds m0-3 (i,f), z_go holds m4-7 (g,o) so
    sigmoid(i,f) only waits on the first 8 recurrent matmuls.
  - start=True zeroes the containing 2KB PSUM bank; slots pair up (2j, 2j+1)
    per bank, so exactly one writer per bank carries start=True (first
    m-chunk, k=0, even step).
  - zx = Wk^T x_t precomputed off the critical path: step t emits the 16
    64-col zx matmuls for step t+ZX_LEAD, whose slot was last read at step
    t-2 -> the in-order PE queue never stalls on the WAR dep.
  - PE_HAM clock gate: the PE runs at 1.2 GHz unless near-continuously busy
    (then 2.4 GHz). Dummy 512-col matmuls into 2 scratch PSUM banks fill the
    gate-chain window each step, locking K=8/8 for the whole loop.
  - Critical chain per step (~2.5us, at per-op floors): 8 rec mm ->
    sigmoid(i,f) -> tanh(g) -> DVE [tm2 = sig_f*c, tm1 = sig_i*tanh_g,
    c = tm2+tm1] -> tanh(c) -> h = sig_o*tanh_c -> next rec mms.
"""

import os
os.environ.setdefault("TILE_EXHAUSTIVE_MEMORY_SHARE_CHECK", "1")

import numpy as np
import ml_dtypes

import concourse.bass as bass
import concourse.mybir as mybir
import concourse.tile as tile
from concourse import bacc
from concourse.bass_utils import run_bass_kernel_spmd

BF16 = mybir.dt.bfloat16
F32 = mybir.dt.float32
AF = mybir.ActivationFunctionType

NC = 8          # cores
B = 512
BC = B // NC    # 64 batch per core
T = 255         # real steps
TP = 256        # padded steps
U = 256
DIN = 256

Z_SLOTS = 6      # z PSUM slots in rotation (pairs share a 2KB bank)
ZX_LEAD = Z_SLOTS - 2   # zx for step t+ZX_LEAD emitted at step t
FILLER_MM = 6    # 512-col dummy matmuls per step into scratch PSUM

# packed bf16 constant block: column offsets in t_pkb [128, PKB_COLS]
_PK = {}
_off = 0
for _name, _w in [("mot", BC), ("rob", BC), ("re", BC), ("im", BC),
                  ("wm", 256), ("wr", 256), ("wre", 128), ("wim", 128),
                  ("wc", 6 * 256), ("wo", 2), ("blw", 128), ("ind", 512)]:
    _PK[_name] = _off
    _off += _w
PKB_COLS = _off


def build_nc(use_bias_mm=True):
    nc = bacc.Bacc(None, target_bir_lowering=False)

    d_pkb = nc.dram_tensor("pkb", [128, PKB_COLS], BF16, kind="ExternalInput")
    d_pkf = nc.dram_tensor("pkf", [128, 9], F32, kind="ExternalInput")
    d_seq = nc.dram_tensor("seq", [2, 128, TP * BC], BF16, kind="ExternalInput")
    d_wk = nc.dram_tensor("wk", [128, 2, 1024], BF16, kind="ExternalInput")
    d_wrk = nc.dram_tensor("wrk", [128, 2, 1024], BF16, kind="ExternalInput")
    d_y = nc.dram_tensor("y", [1, BC], F32, kind="ExternalOutput")

    with tile.TileContext(nc) as tc:
        with (
            tc.tile_pool(name="sb", bufs=1) as sb,
            tc.tile_pool(name="rot", bufs=3) as rot,
        ):
            t_pkb = sb.tile([128, PKB_COLS], BF16, tag="pkb")
            t_pkf = sb.tile([128, 9], F32, tag="pkf")
            t_wk = sb.tile([128, 2, 1024], BF16, tag="wk")
            t_wrk = sb.tile([128, 2, 1024], BF16, tag="wrk")
            t_seq0 = sb.tile([128, TP * BC], BF16, tag="seq0")
            t_seq1 = sb.tile([128, TP * BC], BF16, tag="seq1")
            t_h = sb.tile([128, 2 * BC], BF16, tag="h")   # h^T (chunk k at cols k*64)
            t_c = sb.tile([128, 2 * BC], BF16, tag="c")   # c^T
            t_cat = sb.tile([128, 6, BC], BF16, tag="cat")
            t_y = sb.tile([1, BC], F32, tag="y")
            t_dm = sb.tile([1, 1], F32, tag="dm")

            def pkb(name, w, rows=128):
                c = _PK[name]
                return t_pkb[0:rows, c:c + w]

            nc.sync.dma_start(t_pkb[:], d_pkb[:])
            nc.sync.dma_start(t_pkf[:], d_pkf[:])
            nc.sync.dma_start(t_wk[:], d_wk[:])
            nc.sync.dma_start(t_wrk[:], d_wrk[:])
            CH = 64 * BC
            for ch in range(TP // 64):
                nc.sync.dma_start(
                    t_seq0[:, ch * CH:(ch + 1) * CH], d_seq[0, :, ch * CH:(ch + 1) * CH])
                nc.sync.dma_start(
                    t_seq1[:, ch * CH:(ch + 1) * CH], d_seq[1, :, ch * CH:(ch + 1) * CH])
            t_seq = [t_seq0, t_seq1]
            # trigger the sigmoid/tanh ACT table load during the DMA phase
            # (the only table set used: every relu below runs on DVE)
            nc.scalar.activation(t_dm[:], t_pkf[0:1, 0:1], AF.Sigmoid)

            # ---- front-end branch MLPs -> state -> h0, c0 ----
            with tc.tile_pool(name="fp", bufs=1, space="PSUM") as fp:
                p6 = fp.tile([128, 6, BC], F32, tag="p6")
                for m in range(2):
                    nc.tensor.matmul(p6[:, m, :],
                                     pkb("wm", 256, 64)[:, m * 128:(m + 1) * 128],
                                     pkb("mot", BC, 64), start=True, stop=True)
                for m in range(2):
                    nc.tensor.matmul(p6[:, 2 + m, :],
                                     pkb("wr", 256)[:, m * 128:(m + 1) * 128],
                                     pkb("rob", BC), start=True, stop=True)
                nc.tensor.matmul(p6[:, 4, :], pkb("wre", 128), pkb("re", BC),
                                 start=True, stop=True)
                nc.tensor.matmul(p6[:, 5, :], pkb("wim", 128), pkb("im", BC),
                                 start=True, stop=True)
                A = mybir.AluOpType
                fsc = fp.tile([128, 2, 512], F32, tag="fsc")
                for j in range(12):
                    nc.tensor.matmul(fsc[:, j % 2, :],
                                     t_pkb[:, 256:384], t_pkb[:, 0:512],
                                     start=True, stop=True,
                                     skip_group_check=True)
                for m in range(6):
                    nc.vector.tensor_scalar(t_cat[:, m, :], p6[:, m, :],
                                            t_pkf[:, m:m + 1], 0.0,
                                            A.add, A.max)
                pst = fp.tile([128, 2, BC], F32, tag="pst")
                for mo in range(2):
                    for kc in range(6):
                        nc.tensor.matmul(
                            pst[:, mo, :],
                            pkb("wc", 1536)[:, kc * 256 + mo * 128:kc * 256 + (mo + 1) * 128],
                            t_cat[:, kc, :],
                            start=(kc == 0), stop=(kc == 5))
                for j in range(12):
                    nc.tensor.matmul(fsc[:, j % 2, :],
                                     t_pkb[:, 256:384], t_pkb[:, 0:512],
                                     start=True, stop=True,
                                     skip_group_check=True)
                for mo in range(2):
                    nc.vector.tensor_scalar(t_h[:, mo * BC:(mo + 1) * BC],
                                            pst[:, mo, :],
                                            t_pkf[:, 6 + mo:7 + mo], 0.0,
                                            A.add, A.max)
                    nc.vector.tensor_scalar(t_c[:, mo * BC:(mo + 1) * BC],
                                            pst[:, mo, :],
                                            t_pkf[:, 6 + mo:7 + mo], 0.0,
                                            A.add, A.max)

            # ---- LSTM recurrence ----
            with tc.tile_pool(name="zp", bufs=1, space="PSUM") as zp:
                z_if = zp.tile([128, Z_SLOTS, 4 * BC], F32, tag="zif")
                z_go = zp.tile([128, Z_SLOTS, 4 * BC], F32, tag="zgo")
                scr = zp.tile([128, 2, 512], F32, tag="scr")

                def zslice(bk, m):
                    zt = z_if if m < 4 else z_go
                    mm = m % 4
                    return zt[:, bk, mm * BC:(mm + 1) * BC]

                def emit_bias1(s):
                    bk = s % Z_SLOTS
                    st = (s % 2 == 0)
                    nc.tensor.matmul(z_if[:, bk, :], pkb("blw", 128, 8),
                                     pkb("ind", 512, 8)[:, 0:256],
                                     start=st, stop=False, skip_group_check=True)
                    nc.tensor.matmul(z_go[:, bk, :], pkb("blw", 128, 8),
                                     pkb("ind", 512, 8)[:, 256:512],
                                     start=st, stop=False, skip_group_check=True)

                def emit_zx1(s, ms):
                    bk = s % Z_SLOTS
                    for m in ms:
                        for k in range(2):
                            st = ((not use_bias_mm) and k == 0
                                  and m in (0, 4) and s % 2 == 0)
                            nc.tensor.matmul(
                                zslice(bk, m),
                                t_wk[:, k, m * 128:(m + 1) * 128],
                                t_seq[k][:, s * BC:(s + 1) * BC],
                                start=st, stop=False,
                                skip_group_check=True)

                def emit_step(t):
                    bk = t % Z_SLOTS
                    # i,f m-chunks first so sigmoid(i,f) fires after 8 matmuls
                    for m in range(4):
                        for k in range(2):
                            nc.tensor.matmul(
                                zslice(bk, m),
                                t_wrk[:, k, m * 128:(m + 1) * 128],
                                t_h[:, k * BC:(k + 1) * BC],
                                start=False,
                                stop=(k == 1 and m == 3),
                                skip_group_check=True)
                    for m in range(4, 8):
                        for k in range(2):
                            nc.tensor.matmul(
                                zslice(bk, m),
                                t_wrk[:, k, m * 128:(m + 1) * 128],
                                t_h[:, k * BC:(k + 1) * BC],
                                start=False,
                                stop=(k == 1 and m == 7),
                                skip_group_check=True)
                    gb = rot.tile([128, 512], BF16, tag="gb")
                    tm1 = rot.tile([128, 128], BF16, tag="tm1")
                    tm2 = rot.tile([128, 128], BF16, tag="tm2")
                    tmc = rot.tile([128, 128], BF16, tag="tmc")
                    nc.scalar.activation(gb[:, 0:256], z_if[:, bk, :], AF.Sigmoid)
                    nc.scalar.activation(gb[:, 256:384], z_go[:, bk, 0:128], AF.Tanh)
                    nc.scalar.activation(gb[:, 384:512], z_go[:, bk, 128:256], AF.Sigmoid)
                    # c = sig(f)*c + sig(i)*tanh(g)
                    nc.vector.tensor_mul(tm2[:], gb[:, 128:256], t_c[:])
                    nc.vector.tensor_mul(tm1[:], gb[:, 0:128], gb[:, 256:384])
                    nc.vector.tensor_add(t_c[:], tm2[:], tm1[:])
                    nc.scalar.activation(tmc[:], t_c[:], AF.Tanh)
                    nc.vector.tensor_mul(t_h[:], gb[:, 384:512], tmc[:])

                # preload the zx pipeline
                for s in range(ZX_LEAD):
                    if use_bias_mm:
                        emit_bias1(s)
                    emit_zx1(s, range(8))
                for t in range(T):
                    emit_step(t)
                    # trickle zx for step t+ZX_LEAD (slot read at step t-2)
                    s = t + ZX_LEAD
                    if s < TP:
                        if use_bias_mm:
                            emit_bias1(s)
                        emit_zx1(s, range(8))
                    # PE_HAM un-throttles only under sustained MM activity;
                    # dummy 512-col matmuls into scratch PSUM fill the
                    # gate-chain window so the PE holds K=8/8 (2.4 GHz).
                    for j in range(FILLER_MM):
                        nc.tensor.matmul(scr[:, j % 2, :],
                                         t_wk[:, 0, 0:128], t_seq0[:, 0:512],
                                         start=True, stop=True,
                                         skip_group_check=True)

            # ---- output head ----
            with tc.tile_pool(name="hp", bufs=1, space="PSUM") as hp:
                py = hp.tile([1, BC], F32, tag="py")
                for k in range(2):
                    nc.tensor.matmul(py[:], pkb("wo", 2)[:, k:k + 1],
                                     t_h[:, k * BC:(k + 1) * BC],
                                     start=(k == 0), stop=(k == 1))
                nc.vector.tensor_scalar(t_y[:], py[:],
                                        t_pkf[0:1, 8:9], 0.0,
                                        mybir.AluOpType.add,
                                        mybir.AluOpType.max)
            nc.sync.dma_start(d_y[:], t_y[:])

    nc.compile()
    return nc


_NC_CACHE = None


def _prep_inputs(inputs):
    """Shard + lay out the full-problem inputs into 8 per-core in_maps."""
    bf = ml_dtypes.bfloat16
    f32 = np.float32

    hist = np.asarray(inputs["history"], f32)     # [B, 128, 256]
    act = np.asarray(inputs["action"], f32)       # [B, 128, 256]
    seq = np.concatenate([hist[:, :127], act], axis=1)          # [B, 255, 256]
    seq = np.concatenate(
        [seq, np.zeros((B, 1, DIN), f32)], axis=1)              # [B, 256, 256]

    Wk = np.asarray(inputs["Wk"], f32)            # [256, 1024]
    Wrk = np.asarray(inputs["Wrk"], f32)
    bl = np.asarray(inputs["bl"], f32)            # [1024]
    wk_p = np.ascontiguousarray(
        Wk.reshape(2, 128, 1024).transpose(1, 0, 2)).astype(bf)   # [128,2,1024]
    wrk_p = np.ascontiguousarray(
        Wrk.reshape(2, 128, 1024).transpose(1, 0, 2)).astype(bf)

    # packed bf16 constants (shared across cores except the 4 input slices)
    pkb = np.zeros((128, PKB_COLS), f32)

    def put(name, arr, rows=None):
        a = np.asarray(arr, f32)
        r, w = a.shape
        pkb[0:r, _PK[name]:_PK[name] + w] = a

    put("wm", inputs["Wm"])
    put("wr", inputs["Wr"])
    put("wre", inputs["Wre"])
    put("wim", inputs["Wim"])
    Wc = np.asarray(inputs["Wc"], f32)            # [768, 256] -> [128, 6*256]
    put("wc", np.ascontiguousarray(
        Wc.reshape(6, 128, 256).transpose(1, 0, 2)).reshape(128, 1536))
    Wo = np.asarray(inputs["Wo"], f32)            # [256, 1] -> [128, 2]
    put("wo", np.ascontiguousarray(Wo.reshape(2, 128).T))
    put("blw", bl.reshape(8, 128))
    ind8 = np.zeros((8, 512), f32)
    for j in range(8):
        ind8[j, j * 64:(j + 1) * 64] = 1.0
    put("ind", ind8)

    # packed f32 biases: [bm(2), br(2), bre(1), bim(1), bc(2), bo(1)]
    pkf = np.zeros((128, 9), f32)
    pkf[:, 0:2] = np.asarray(inputs["bm"], f32).reshape(2, 128).T
    pkf[:, 2:4] = np.asarray(inputs["br"], f32).reshape(2, 128).T
    pkf[:, 4:5] = np.asarray(inputs["bre"], f32).reshape(1, 128).T
    pkf[:, 5:6] = np.asarray(inputs["bim"], f32).reshape(1, 128).T
    pkf[:, 6:8] = np.asarray(inputs["bc"], f32).reshape(2, 128).T
    pkf[0, 8] = np.asarray(inputs["bo"], f32).ravel()[0]

    mot = np.asarray(inputs["motion_state"], f32)
    rob = np.asarray(inputs["robot_state"], f32)
    real = np.concatenate([np.asarray(inputs["osc_state_real"], f32),
                           np.asarray(inputs["osc_real"], f32)], -1)
    imag = np.concatenate([np.asarray(inputs["osc_state_imag"], f32),
                           np.asarray(inputs["osc_imag"], f32)], -1)

    in_maps = []
    for c in range(NC):
        sl = slice(c * BC, (c + 1) * BC)
        # on-chip col = t*64 + b  (plain t-major)
        sc = seq[sl].reshape(BC, TP, 2, 128)           # [b, t, fk, fp]
        sc = np.ascontiguousarray(sc.transpose(2, 3, 1, 0)).astype(bf)
        pk = pkb.copy()
        pk[0:64, _PK["mot"]:_PK["mot"] + BC] = mot[sl].T
        pk[:, _PK["rob"]:_PK["rob"] + BC] = rob[sl].T
        pk[:, _PK["re"]:_PK["re"] + BC] = real[sl].T
        pk[:, _PK["im"]:_PK["im"] + BC] = imag[sl].T
        m = {
            "pkb": pk.astype(bf),
            "pkf": pkf,
            "wk": wk_p, "wrk": wrk_p,
            "seq": np.ascontiguousarray(sc.reshape(2, 128, TP * BC)),
        }
        in_maps.append(m)
    return in_maps


def kernel(**inputs):
    global _NC_CACHE
    use_bias_mm = bool(np.any(np.asarray(inputs["bl"])))
    if _NC_CACHE is None or _NC_CACHE[1] != use_bias_mm:
        _NC_CACHE = (build_nc(use_bias_mm), use_bias_mm)
    in_maps = _prep_inputs(inputs)
    res = run_bass_kernel_spmd(_NC_CACHE[0], in_maps, core_ids=list(range(NC)))
    out = np.concatenate(
        [np.asarray(res.results[c]["y"], np.float32).T for c in range(NC)], axis=0)
    return out  # [512, 1] float32
